# revision 16
# baseline (speedup 1.0000x reference)
"""Edge dot-product kernel for 8 trn2 NeuronCores.

score[e] = <h_src[src_idx[e]], h_dst[dst_idx[e]]>, E=625000, D=128, N=100000.

Design:
- Edges sharded by SRC ROW RANGE: core c owns edges with src_idx in
  [c*12500, (c+1)*12500). Each core sorts its edges by (dst_quarter,
  src_row) and packs them into tiles of <=128 edges whose src rows span
  < 128 (a "window" [r0, r0+128) of the src table slice).
- SRC side (PE): per tile, a host-built one-hot selection matrix
  oh[r, e] = (u_e == r0 + r) is DMA'd and matmul'd with the host-sliced
  window rows (lhsT=oh [128r x 128e], rhs=wnd [128r x 128d]) giving
  exact-f32 gathered src rows in PSUM [e, d].
- DST side (Q7 SWDGE): dma_gather per 64-tile chunk. This is the
  bottleneck engine (~8ns/idx descriptor generation, ~630us busy); all
  PE/DVE/DMA work is sized to hide under it.
- DVE: prod = psum_s * t (per 4-tile PSUM bank), tensor_reduce over d
  -> scores [e_slot, tile]. Host inverse-permutes.
"""

import numpy as np

N = 100000
D = 128
E = 625000
NCORES = 8
P = 128
TPC = 32          # tiles per dma_gather call
BG = 4            # tiles per PSUM bank group

_cache = {}


def _build_bass(GT, calls, QD):
    """GT: tiles per dst-quarter group; calls: tuple of (group, t0, ntiles)."""
    import concourse.bacc as bacc
    import concourse.tile as tile
    from concourse import mybir

    f32 = mybir.dt.float32
    T4 = 4 * GT

    nc = bacc.Bacc("TRN2", target_bir_lowering=False)

    wnd = nc.dram_tensor("wnd", [T4, P, D], f32, kind="ExternalInput")
    ohk = nc.dram_tensor("ohk", [T4, P, P], f32, kind="ExternalInput")
    dq = nc.dram_tensor("dq", [P, T4 * (P // 16)], mybir.dt.int16,
                        kind="ExternalInput")
    hd = [nc.dram_tensor(f"hd{q}", [QD, D], f32, kind="ExternalInput")
          for q in range(4)]
    scores = nc.dram_tensor("scores", [P, T4], f32, kind="ExternalOutput")

    CW = P // 16  # int16 idx words per partition per tile

    with tile.TileContext(nc) as tc:
        with (
            tc.tile_pool(name="const", bufs=1) as constp,
            tc.tile_pool(name="tp", bufs=4) as tp,
            tc.tile_pool(name="wp", bufs=6) as wp,
            tc.tile_pool(name="ohp", bufs=6) as ohp,
            tc.tile_pool(name="prodp", bufs=4) as prodp,
            tc.tile_pool(name="accp", bufs=1) as accp,
            tc.tile_pool(name="psa", bufs=6, space="PSUM") as psa,
        ):
            # idx array loaded upfront in two separate tiles: a small one
            # covering the first gather call (so it can start immediately,
            # without a dependency on the big transfer) + the rest
            (g0, t00, nt0) = calls[0]
            cut = (g0 * GT + t00 + nt0) * CW
            it0 = constp.tile([P, cut], mybir.dt.int16, tag="i0")
            nc.sync.dma_start(out=it0[:], in_=dq[:, :cut])
            it = constp.tile([P, T4 * CW - cut], mybir.dt.int16, tag="i")
            nc.sync.dma_start(out=it[:], in_=dq[:, cut:])
            sc = accp.tile([P, T4], f32, tag="sc")
            # (no warmup gather: the ext-isa library load attaches to the
            # first extended instruction regardless, and the first real
            # gather's idx slice lands well before the load completes)
            for ci, (g, t0, nt) in enumerate(calls):
                C = nt * P
                gbase = g * GT + t0
                if ci == 0:
                    idx_src = it0[:, :nt * CW]
                else:
                    idx_src = it[:, gbase * CW - cut:(gbase + nt) * CW - cut]
                tt = tp.tile([P, TPC * D], f32, tag="t")
                nc.gpsimd.dma_gather(
                    out_ap=tt[:, :nt * D].rearrange("p (c d) -> p c d", d=D),
                    in_ap=hd[g][:],
                    idxs_ap=idx_src,
                    num_idxs=C,
                    num_idxs_reg=C,
                    elem_size=D,
                    single_packet=False,
                )
                for b0 in range(0, nt, BG):
                    nb = min(BG, nt - b0)
                    F = nb * P
                    tb = gbase + b0
                    wt = wp.tile([P, BG * D], f32, tag="w")
                    nc.sync.dma_start(
                        out=wt[:, :F].rearrange("p (k d) -> p k d", d=D),
                        in_=wnd[tb:tb + nb].rearrange("k p d -> p k d"),
                    )
                    oh = ohp.tile([P, BG * P], f32, tag="oh")
                    # scalar engine = the second HWDGE ring; keeps one-hot
                    # loads off the sync ring feeding wnd
                    nc.scalar.dma_start(
                        out=oh[:, :F].rearrange("p (k e) -> p k e", e=P),
                        in_=ohk[tb:tb + nb].rearrange("k r e -> r k e"),
                    )
                    psa_t = psa.tile([P, BG * D], f32, tag="pa")
                    for k in range(nb):
                        nc.tensor.matmul(
                            out=psa_t[:, k * D:(k + 1) * D],
                            lhsT=oh[:, k * P:(k + 1) * P],
                            rhs=wt[:, k * D:(k + 1) * D],
                            start=True,
                            stop=True,
                        )
                    pr = prodp.tile([P, BG * D], f32, tag="pr")
                    nc.vector.tensor_tensor(
                        out=pr[:, :F],
                        in0=psa_t[:, :F],
                        in1=tt[:, b0 * D:b0 * D + F],
                        op=mybir.AluOpType.mult,
                    )
                    nc.vector.tensor_reduce(
                        out=sc[:, tb:tb + nb],
                        in_=pr[:, :F].rearrange("p (c d) -> p c d", d=D),
                        axis=mybir.AxisListType.X,
                        op=mybir.AluOpType.add,
                    )
                # flush this quarter's scores as soon as it completes so the
                # final write-out isn't serialized into the kernel tail
                if t0 + nt == GT:
                    nc.sync.dma_start(
                        out=scores[:, g * GT:(g + 1) * GT],
                        in_=sc[:, g * GT:(g + 1) * GT],
                    )
    nc.finalize()
    return nc


def _wrap16(vals, C):
    """int16 stream [C] -> [P, C//16] tile (16-way wrap, replicated x8)."""
    w16 = vals.reshape(C // 16, 16).T
    return np.tile(w16, (8, 1))


def _prepare(h_src, h_dst, src_idx, dst_idx, ncores=NCORES, n=N, e=E):
    h_src = np.ascontiguousarray(np.asarray(h_src, dtype=np.float32))
    h_dst = np.ascontiguousarray(np.asarray(h_dst, dtype=np.float32))
    src_idx = np.asarray(src_idx).astype(np.int64)
    dst_idx = np.asarray(dst_idx).astype(np.int64)

    src_sh = n // ncores
    qd = n // 4

    core_of = src_idx // src_sh
    # pass 1: per-core sorted edge arrays and per-group tile boundary lists
    per_core = []
    gt_max = 0
    for c in range(ncores):
        eids = np.nonzero(core_of == c)[0]
        u = (src_idx[eids] - c * src_sh).astype(np.int64)
        v = dst_idx[eids]
        g = v // qd
        order = np.lexsort((u, g))
        u, v, g, eids = u[order], v[order], g[order], eids[order]
        groups = []
        for gq in range(4):
            lo = np.searchsorted(g, gq, side="left")
            hi = np.searchsorted(g, gq, side="right")
            # tile boundaries within [lo, hi): <=128 edges, src span < 128
            bounds = []
            start = lo
            while start < hi:
                r0 = int(u[start])
                end_span = lo + int(np.searchsorted(u[lo:hi], r0 + P, side="left"))
                end = min(start + P, end_span, hi)
                bounds.append((start, end, r0))
                start = end
            groups.append(bounds)
            gt_max = max(gt_max, len(bounds))
        per_core.append((eids, u, v, groups))

    GT = gt_max  # BG loop handles ragged tails; no rounding needed
    T4 = 4 * GT
    calls = []
    for gq in range(4):
        t0 = 0
        while t0 < GT:
            nt = min(TPC, GT - t0)
            # taper the kernel's final gather calls so the compute tail
            # exposed after the last descriptor generation stays short
            if gq == 3 and t0 + nt == GT and nt > 3 * BG:
                a = nt - 3 * BG
                calls.append((gq, t0, a))
                calls.append((gq, t0 + a, 2 * BG))
                calls.append((gq, t0 + a + 2 * BG, BG))
            else:
                calls.append((gq, t0, nt))
            t0 += nt
    calls = tuple(calls)

    key = (GT, calls, qd)
    if key not in _cache:
        _cache[key] = _build_bass(GT, calls, qd)
    nc = _cache[key]

    hdq = [np.ascontiguousarray(h_dst[q * qd:(q + 1) * qd]) for q in range(4)]

    CW = P // 16
    in_maps = []
    meta_cores = []
    for c in range(ncores):
        eids, u, v, groups = per_core[c]
        wnd = np.zeros((T4, P, D), dtype=np.float32)
        ohk = np.zeros((T4, P, P), dtype=np.float32)
        # pad slots use idx 0 (valid row): the Q7 generates descriptors for
        # them (~2% overhead) but num_idxs_reg stays the static padded count,
        # which both the interp contract and SPMD (per-core counts differ)
        # require. Pad one-hot columns are all-zero -> psum row 0 -> score
        # garbage dropped by host.
        dqi = np.zeros((T4 * P,), dtype=np.int16)
        slot = np.empty(len(u), dtype=np.int64)  # slot = col*P + partition
        hs = h_src[c * src_sh:(c + 1) * src_sh]
        for gq in range(4):
            for j, (st, en, r0) in enumerate(groups[gq]):
                col = gq * GT + j
                cnt = en - st
                rows = min(P, src_sh - r0)
                wnd[col, :rows] = hs[r0:r0 + rows]
                ohk[col, u[st:en] - r0, np.arange(cnt)] = 1.0
                base = col * P
                dqi[base:base + cnt] = (v[st:en] - gq * qd).astype(np.int16)
                slot[st:en] = base + np.arange(cnt)
        # wrap idx stream per call
        dqw = np.zeros((P, T4 * CW), dtype=np.int16)
        for (gq, t0, nt) in calls:
            gbase = gq * GT + t0
            seg = dqi[gbase * P:(gbase + nt) * P]
            dqw[:, gbase * CW:(gbase + nt) * CW] = _wrap16(seg, nt * P)
        im = {"wnd": wnd, "ohk": ohk, "dq": dqw}
        for q in range(4):
            im[f"hd{q}"] = hdq[q]
        in_maps.append(im)
        meta_cores.append((eids, slot))

    meta = {"cores": meta_cores, "T4": T4, "e": e}
    return nc, in_maps, meta


def _postprocess(results, meta):
    out = np.empty(meta["e"], dtype=np.float32)
    for c, (eids, slot) in enumerate(meta["cores"]):
        sc = results[c]["scores"]               # [P, T4]
        flat = sc.T.reshape(-1)                 # slot = col*P + partition
        out[eids] = flat[slot]
    return out.reshape(-1, 1)


LAST_RESULTS = None


def kernel(h_src, h_dst, src_idx, dst_idx):
    global LAST_RESULTS
    from concourse.bass_utils import run_bass_kernel_spmd

    nc, in_maps, meta = _prepare(h_src, h_dst, src_idx, dst_idx)
    res = run_bass_kernel_spmd(nc, in_maps, core_ids=list(range(NCORES)))
    LAST_RESULTS = res
    return _postprocess(res.results, meta)


# revision 17
# speedup vs baseline: 1.1853x; 1.1853x over previous
"""Edge dot-product kernel for 8 trn2 NeuronCores.

score[e] = <h_src[src_idx[e]], h_dst[dst_idx[e]]>, E=625000, D=128, N=100000.

Design:
- Edges sharded by SRC ROW RANGE: core c owns edges with src_idx in
  [c*12500, (c+1)*12500). Each core sorts its edges by (dst_quarter,
  src_row) and packs them into tiles of <=128 edges whose src rows span
  < 128 (a "window" [r0, r0+128) of the src table slice).
- SRC side (PE): per tile, a host-built one-hot selection matrix
  oh[r, e] = (u_e == r0 + r) is DMA'd and matmul'd with the host-sliced
  window rows (lhsT=oh [128r x 128e], rhs=wnd [128r x 128d]) giving
  exact-f32 gathered src rows in PSUM [e, d].
- DST side (Q7 SWDGE): dma_gather per 64-tile chunk. This is the
  bottleneck engine (~8ns/idx descriptor generation, ~630us busy); all
  PE/DVE/DMA work is sized to hide under it.
- DVE: prod = psum_s * t (per 4-tile PSUM bank), tensor_reduce over d
  -> scores [e_slot, tile]. Host inverse-permutes.
"""

import numpy as np

N = 100000
D = 128
E = 625000
NCORES = 8
P = 128
TPC = 32          # tiles per dma_gather call
BG = 4            # tiles per PSUM bank group

_cache = {}


def _build_bass(GT, calls, QD):
    """GT: tiles per dst-quarter group; calls: tuple of (group, t0, ntiles)."""
    import concourse.bacc as bacc
    import concourse.tile as tile
    from concourse import mybir

    f32 = mybir.dt.float32
    T4 = 4 * GT

    nc = bacc.Bacc("TRN2", target_bir_lowering=False)

    wnd = nc.dram_tensor("wnd", [T4, P, D], f32, kind="ExternalInput")
    ohk = nc.dram_tensor("ohk", [T4, P, P], f32, kind="ExternalInput")
    dq = nc.dram_tensor("dq", [P, T4 * (P // 16)], mybir.dt.int16,
                        kind="ExternalInput")
    hd = [nc.dram_tensor(f"hd{q}", [QD, D], f32, kind="ExternalInput")
          for q in range(4)]
    scores = nc.dram_tensor("scores", [P, T4], f32, kind="ExternalOutput")

    CW = P // 16  # int16 idx words per partition per tile

    with tile.TileContext(nc) as tc:
        with (
            tc.tile_pool(name="const", bufs=1) as constp,
            tc.tile_pool(name="tp", bufs=4) as tp,
            tc.tile_pool(name="wp", bufs=6) as wp,
            tc.tile_pool(name="ohp", bufs=6) as ohp,
            tc.tile_pool(name="prodp", bufs=4) as prodp,
            tc.tile_pool(name="accp", bufs=1) as accp,
            tc.tile_pool(name="psa", bufs=6, space="PSUM") as psa,
        ):
            # idx array loaded upfront in two separate tiles: a small one
            # covering the first gather call (so it can start immediately,
            # without a dependency on the big transfer) + the rest
            (g0, t00, nt0) = calls[0]
            cut = (g0 * GT + t00 + nt0) * CW
            it0 = constp.tile([P, cut], mybir.dt.int16, tag="i0")
            nc.sync.dma_start(out=it0[:], in_=dq[:, :cut])
            it = constp.tile([P, T4 * CW - cut], mybir.dt.int16, tag="i")
            nc.sync.dma_start(out=it[:], in_=dq[:, cut:])
            sc = accp.tile([P, T4], f32, tag="sc")
            # warmup: a tiny gather over the first call's already-loaded idx
            # slice absorbs the ext-isa IRAM load; output is discarded. No
            # Pool-engine work precedes it, so the library load starts as
            # early as possible.
            ttw = constp.tile([P, D], f32, tag="tw")
            nc.gpsimd.dma_gather(
                out_ap=ttw[:].rearrange("p (c d) -> p c d", d=D),
                in_ap=hd[calls[0][0]][:],
                idxs_ap=it0[:, :CW],
                num_idxs=P,
                num_idxs_reg=P,
                elem_size=D,
                single_packet=False,
            )
            for ci, (g, t0, nt) in enumerate(calls):
                C = nt * P
                gbase = g * GT + t0
                if ci == 0:
                    idx_src = it0[:, :nt * CW]
                else:
                    idx_src = it[:, gbase * CW - cut:(gbase + nt) * CW - cut]
                tt = tp.tile([P, TPC * D], f32, tag="t")
                nc.gpsimd.dma_gather(
                    out_ap=tt[:, :nt * D].rearrange("p (c d) -> p c d", d=D),
                    in_ap=hd[g][:],
                    idxs_ap=idx_src,
                    num_idxs=C,
                    num_idxs_reg=C,
                    elem_size=D,
                    single_packet=False,
                )
                for b0 in range(0, nt, BG):
                    nb = min(BG, nt - b0)
                    F = nb * P
                    tb = gbase + b0
                    wt = wp.tile([P, BG * D], f32, tag="w")
                    nc.sync.dma_start(
                        out=wt[:, :F].rearrange("p (k d) -> p k d", d=D),
                        in_=wnd[tb:tb + nb].rearrange("k p d -> p k d"),
                    )
                    oh = ohp.tile([P, BG * P], f32, tag="oh")
                    # scalar engine = the second HWDGE ring; keeps one-hot
                    # loads off the sync ring feeding wnd
                    nc.scalar.dma_start(
                        out=oh[:, :F].rearrange("p (k e) -> p k e", e=P),
                        in_=ohk[tb:tb + nb].rearrange("k r e -> r k e"),
                    )
                    psa_t = psa.tile([P, BG * D], f32, tag="pa")
                    for k in range(nb):
                        nc.tensor.matmul(
                            out=psa_t[:, k * D:(k + 1) * D],
                            lhsT=oh[:, k * P:(k + 1) * P],
                            rhs=wt[:, k * D:(k + 1) * D],
                            start=True,
                            stop=True,
                        )
                    pr = prodp.tile([P, BG * D], f32, tag="pr")
                    nc.vector.tensor_tensor(
                        out=pr[:, :F],
                        in0=psa_t[:, :F],
                        in1=tt[:, b0 * D:b0 * D + F],
                        op=mybir.AluOpType.mult,
                    )
                    nc.vector.tensor_reduce(
                        out=sc[:, tb:tb + nb],
                        in_=pr[:, :F].rearrange("p (c d) -> p c d", d=D),
                        axis=mybir.AxisListType.X,
                        op=mybir.AluOpType.add,
                    )
                # flush this quarter's scores as soon as it completes so the
                # final write-out isn't serialized into the kernel tail
                if t0 + nt == GT:
                    nc.sync.dma_start(
                        out=scores[:, g * GT:(g + 1) * GT],
                        in_=sc[:, g * GT:(g + 1) * GT],
                    )
    nc.finalize()
    return nc


def _wrap16(vals, C):
    """int16 stream [C] -> [P, C//16] tile (16-way wrap, replicated x8)."""
    w16 = vals.reshape(C // 16, 16).T
    return np.tile(w16, (8, 1))


def _prepare(h_src, h_dst, src_idx, dst_idx, ncores=NCORES, n=N, e=E):
    h_src = np.ascontiguousarray(np.asarray(h_src, dtype=np.float32))
    h_dst = np.ascontiguousarray(np.asarray(h_dst, dtype=np.float32))
    src_idx = np.asarray(src_idx).astype(np.int64)
    dst_idx = np.asarray(dst_idx).astype(np.int64)

    src_sh = n // ncores
    qd = n // 4

    core_of = src_idx // src_sh
    # pass 1: per-core sorted edge arrays and per-group tile boundary lists
    per_core = []
    gt_max = 0
    for c in range(ncores):
        eids = np.nonzero(core_of == c)[0]
        u = (src_idx[eids] - c * src_sh).astype(np.int64)
        v = dst_idx[eids]
        g = v // qd
        order = np.lexsort((u, g))
        u, v, g, eids = u[order], v[order], g[order], eids[order]
        groups = []
        for gq in range(4):
            lo = np.searchsorted(g, gq, side="left")
            hi = np.searchsorted(g, gq, side="right")
            # tile boundaries within [lo, hi): <=128 edges, src span < 128
            bounds = []
            start = lo
            while start < hi:
                r0 = int(u[start])
                end_span = lo + int(np.searchsorted(u[lo:hi], r0 + P, side="left"))
                end = min(start + P, end_span, hi)
                bounds.append((start, end, r0))
                start = end
            groups.append(bounds)
            gt_max = max(gt_max, len(bounds))
        per_core.append((eids, u, v, groups))

    GT = gt_max  # BG loop handles ragged tails; no rounding needed
    T4 = 4 * GT
    calls = []
    for gq in range(4):
        t0 = 0
        while t0 < GT:
            nt = min(TPC, GT - t0)
            # keep the kernel's very last gather call small so the
            # post-gather compute tail is short
            if gq == 3 and t0 + nt == GT and nt > 2 * BG:
                calls.append((gq, t0, nt - BG))
                calls.append((gq, t0 + nt - BG, BG))
            else:
                calls.append((gq, t0, nt))
            t0 += nt
    calls = tuple(calls)

    key = (GT, calls, qd)
    if key not in _cache:
        _cache[key] = _build_bass(GT, calls, qd)
    nc = _cache[key]

    hdq = [np.ascontiguousarray(h_dst[q * qd:(q + 1) * qd]) for q in range(4)]

    CW = P // 16
    in_maps = []
    meta_cores = []
    for c in range(ncores):
        eids, u, v, groups = per_core[c]
        wnd = np.zeros((T4, P, D), dtype=np.float32)
        ohk = np.zeros((T4, P, P), dtype=np.float32)
        # pad slots use idx 0 (valid row): the Q7 generates descriptors for
        # them (~2% overhead) but num_idxs_reg stays the static padded count,
        # which both the interp contract and SPMD (per-core counts differ)
        # require. Pad one-hot columns are all-zero -> psum row 0 -> score
        # garbage dropped by host.
        dqi = np.zeros((T4 * P,), dtype=np.int16)
        slot = np.empty(len(u), dtype=np.int64)  # slot = col*P + partition
        hs = h_src[c * src_sh:(c + 1) * src_sh]
        for gq in range(4):
            for j, (st, en, r0) in enumerate(groups[gq]):
                col = gq * GT + j
                cnt = en - st
                rows = min(P, src_sh - r0)
                wnd[col, :rows] = hs[r0:r0 + rows]
                ohk[col, u[st:en] - r0, np.arange(cnt)] = 1.0
                base = col * P
                dqi[base:base + cnt] = (v[st:en] - gq * qd).astype(np.int16)
                slot[st:en] = base + np.arange(cnt)
        # wrap idx stream per call
        dqw = np.zeros((P, T4 * CW), dtype=np.int16)
        for (gq, t0, nt) in calls:
            gbase = gq * GT + t0
            seg = dqi[gbase * P:(gbase + nt) * P]
            dqw[:, gbase * CW:(gbase + nt) * CW] = _wrap16(seg, nt * P)
        im = {"wnd": wnd, "ohk": ohk, "dq": dqw}
        for q in range(4):
            im[f"hd{q}"] = hdq[q]
        in_maps.append(im)
        meta_cores.append((eids, slot))

    meta = {"cores": meta_cores, "T4": T4, "e": e}
    return nc, in_maps, meta


def _postprocess(results, meta):
    out = np.empty(meta["e"], dtype=np.float32)
    for c, (eids, slot) in enumerate(meta["cores"]):
        sc = results[c]["scores"]               # [P, T4]
        flat = sc.T.reshape(-1)                 # slot = col*P + partition
        out[eids] = flat[slot]
    return out.reshape(-1, 1)


LAST_RESULTS = None


def kernel(h_src, h_dst, src_idx, dst_idx):
    global LAST_RESULTS
    from concourse.bass_utils import run_bass_kernel_spmd

    nc, in_maps, meta = _prepare(h_src, h_dst, src_idx, dst_idx)
    res = run_bass_kernel_spmd(nc, in_maps, core_ids=list(range(NCORES)))
    LAST_RESULTS = res
    return _postprocess(res.results, meta)
